# revision 49
# baseline (speedup 1.0000x reference)
"""Trainium2 Bass kernel for Transformer-XL style relative-position attention.

Reference computation (B=2, Tq=1024, Tkv=2048, D=1024, H=16, Dv=64):
    q/k/v/r projections, ac = (q+cb)@k^T, bd = rel_shift((q+pb)@r^T),
    softmax((ac+bd)/8) with causal-with-memory mask, ctx = attn@v,
    out = LN(ctx@Wo + query).

Sharding (Megatron-style tensor parallel over heads, 8 cores):
  - each core owns 2 heads: Wq/Wk/Wv/Wr column shards [1024,128], biases
    shard with heads.
  - activations (transposed on host to feature-major) are broadcast.
  - after per-head attention, ctx^T shards are exchanged with a single
    AllToAll so that each core ends up with the full ctx^T for 1/8 of the
    token rows; each core then does that row-slice of ctx@Wo + residual +
    LayerNorm with the full Wo.

Device-side structure (transposed-scores design):
  - scores are computed TRANSPOSED (kv on partitions, q on the free dim):
    acT tiles come straight from a matmul with kT as the stationary
    operand; this makes the softmax output directly consumable by the
    attn@v contraction with NO attention-matrix transpose or DRAM
    round-trip.
  - rel_shift: bd is computed q-major (dense PE work), EXPONENTIATED
    (exp commutes with the shift), written bf16 to a flat DRAM scratch,
    and read back through a strided AP with row stride Tkv-1 PLUS
    transpose=True (hardware XBAR transpose), which lands the *shifted,
    transposed* exp(bd/8) tiles in SBUF in one step.
  - softmax numerator: exp((ac+bd)/8) = exp(ac/8) * exp(bd/8): the
    scalar engine exponentiates acT from PSUM, and the product runs on
    gpsimd/vector (SBUF-only operands), so no engine ever needs an
    (ac+bd) add against PSUM.
  - causal mask applied to the exp(bd) tiles with affine_select fill=0.
  - softmax denominators come for free from a ones-column appended to v:
    the attn@v matmul accumulates sum(exp) in psum row 64.
  - 1/denominator (per q) is broadcast across the 64 feature partitions
    with a rank-1 matmul (ones ⊗ recip) and applied to the small ctx^T
    tile instead of the big attention matrix.
"""

import numpy as np

# problem shapes (hardcoded per contract)
B, TQ, TKV, D, NH, DV = 2, 1024, 2048, 1024, 16, 64
N_CORES = 8
HPC = NH // N_CORES          # heads per core = 2
FPC = HPC * DV               # head-feature columns per core = 128
RPC = (B * TQ) // N_CORES    # output token rows per core = 256
R_OFF = TKV - TQ             # causal memory offset = 1024
LN_EPS = 1e-5
NT = TQ // 128               # query row chunks = 8
NK = TKV // 512              # key col chunks of 512 = 4

_CACHE = {}


def _patched_tc_class():
    """TileContext whose kernel-tail drain splits sem waits one per drain.

    The walrus build in this container rejects CTRL-type instructions
    (InstDrain) carrying more than one sync-wait command.
    """
    import concourse.mybir as mybir
    import concourse.tile as tile
    from concourse.vector_clock import ScopedClock

    class TC(tile.TileContext):
        def _commit_instruction(self, inst, lazy_reg_writes=True):
            # This walrus build rejects instructions carrying more than one
            # sync-wait command; hoist extras onto preceding NoOp carriers.
            si = getattr(inst, "sync_info", None)
            if (
                si is not None
                and si.on_wait
                and len(si.on_wait) > 1
                and inst.engine != mybir.EngineType.Unassigned
            ):
                waits = list(si.on_wait)
                inst.sync_info = mybir.SyncInfo(
                    on_wait=[waits[-1]], on_update=list(si.on_update or [])
                )
                for w in waits[:-1]:
                    ev = mybir.InstNoOp(
                        name=f"I-wsplit-{self.nc.next_id()}", ins=[], outs=[]
                    )
                    ev.engine = inst.engine
                    ev.sync_info = mybir.SyncInfo(on_wait=[w], on_update=[])
                    self._add_instruction(ev)
            return super()._commit_instruction(inst, lazy_reg_writes)

        def _drain_and_barrier(self, tick_clock, wait_clock):
            nc = self.nc
            drain_inst = nc.sync.drain()
            wait_clock.add_sem_waits(
                drain_inst.ins, ScopedClock({None: tick_clock.global_clock})
            )
            inner = drain_inst.ins
            si = inner.sync_info
            waits = list(si.on_wait) if si and si.on_wait else []
            if len(waits) > 1:
                inner.sync_info = mybir.SyncInfo(
                    on_wait=waits[:1], on_update=list(si.on_update or [])
                )
                for w in waits[1:]:
                    d2 = nc.sync.drain()
                    d2.ins.sync_info = mybir.SyncInfo(on_wait=[w], on_update=[])
            nc.all_engine_barrier()
            assert self.sems is not None
            popped = nc._tile_sem_poison_stack.pop()
            assert popped is self._sem_poison
            nc.clear_and_free_semaphores(list(self.sems.allocated().values()))
            nc.all_engine_barrier()

    return TC


def build_program(score_dtype="bfloat16", proj_dtype="bfloat16", n_cores=N_CORES):
    """Build the SPMD Bass program (identical on all 8 cores).

    n_cores=1 builds a single-core variant (collective replaced by a
    self-copy) for profiling; its output is only valid for core 0's
    feature shard.
    """
    import concourse.bass as bass
    import concourse.mybir as mybir
    from concourse.bass import AP

    f32 = mybir.dt.float32
    bf16 = mybir.dt.bfloat16
    f8 = mybir.dt.float8e4
    DR = mybir.MatmulPerfMode.DoubleRow
    pdt = bf16
    sdt = bf16
    TC = _patched_tc_class()

    nc = bass.Bass()

    # ---- I/O ----
    xqT = nc.dram_tensor("xqT", [D, B * TQ], f8, kind="ExternalInput")
    xkvT = nc.dram_tensor("xkvT", [D, B * TKV], f8, kind="ExternalInput")
    xrT = nc.dram_tensor("xrT", [D, B * TKV], f8, kind="ExternalInput")
    wq = nc.dram_tensor("wq", [D, FPC], f8, kind="ExternalInput")
    wk = nc.dram_tensor("wk", [D, FPC], f8, kind="ExternalInput")
    wv = nc.dram_tensor("wv", [D, FPC], f8, kind="ExternalInput")
    wr = nc.dram_tensor("wr", [D, FPC], f8, kind="ExternalInput")
    wo = nc.dram_tensor("wo", [D, D], pdt, kind="ExternalInput")
    cbv = nc.dram_tensor("cbv", [FPC, 1], f32, kind="ExternalInput")
    pbv = nc.dram_tensor("pbv", [FPC, 1], f32, kind="ExternalInput")
    qres = nc.dram_tensor("qres", [RPC, D], f32, kind="ExternalInput")
    gamma = nc.dram_tensor("gamma", [D], f32, kind="ExternalInput")
    beta = nc.dram_tensor("beta", [D], f32, kind="ExternalInput")
    out = nc.dram_tensor("out", [RPC, D], f32, kind="ExternalOutput")

    # ---- internal DRAM scratch ----
    # raw bd per (pair, q-half): flat [512 rows x TKV]; the shifted,
    # transposed read only ever depends on its own half's rows.
    bd_dram = [
        [nc.dram_tensor(f"bd_dram{p}_{h}", [512 * TKV], bf16) for h in range(2)]
        for p in range(4)
    ]
    # one AllToAll per batch: core c owns q rows [128c, 128c+128) of EACH
    # batch, so batch-0 exchange + output projection overlap batch-1 attention
    a2a_in = [nc.dram_tensor(f"a2a_in{b}", [N_CORES * FPC, TQ // 8], pdt) for b in range(B)]
    a2a_out = [nc.dram_tensor(f"a2a_out{b}", [N_CORES * FPC, TQ // 8], pdt) for b in range(B)]

    Exp = mybir.ActivationFunctionType.Exp
    Identity = mybir.ActivationFunctionType.Identity
    Sqrt = mybir.ActivationFunctionType.Sqrt
    ALU = mybir.AluOpType

    with TC(nc) as tc:
        import contextlib

        with contextlib.ExitStack() as ctx:
            singles = ctx.enter_context(tc.tile_pool(name="singles", bufs=1))

            # ---- static SBUF tensors ----
            wq_sb = singles.tile([128, D // 128, FPC], f8, tag="wq_sb")
            wk_sb = singles.tile([128, D // 128, FPC], f8, tag="wk_sb")
            wv_sb = singles.tile([128, D // 128, FPC], f8, tag="wv_sb")
            wr_sb = singles.tile([128, D // 128, FPC], f8, tag="wr_sb")
            for w_sb, w_dr in ((wq_sb, wq), (wk_sb, wk), (wv_sb, wv), (wr_sb, wr)):
                nc.gpsimd.dma_start(
                    out=w_sb, in_=w_dr[:].rearrange("(kc p) f -> p kc f", p=128)
                )
            wo_sb = singles.tile([128, D // 128, D], pdt, tag="wo_sb")
            nc.gpsimd.dma_start(
                out=wo_sb, in_=wo[:].rearrange("(kc p) d -> p kc d", p=128)
            )
            cb_sb = singles.tile([FPC, 1], f32, tag="cb_sb")
            pb_sb = singles.tile([FPC, 1], f32, tag="pb_sb")
            nc.sync.dma_start(out=cb_sb, in_=cbv[:])
            nc.sync.dma_start(out=pb_sb, in_=pbv[:])
            eps_sb = singles.tile([128, 1], f32, tag="eps_sb")
            nc.vector.memset(eps_sb, LN_EPS)
            gamma_sb = singles.tile([128, D], f32, tag="gamma_sb")
            beta_sb = singles.tile([128, D], f32, tag="beta_sb")
            nc.gpsimd.dma_start(
                out=gamma_sb,
                in_=AP(tensor=gamma[:].tensor, offset=0, ap=[[0, 128], [1, D]]),
            )
            nc.gpsimd.dma_start(
                out=beta_sb,
                in_=AP(tensor=beta[:].tensor, offset=0, ap=[[0, 128], [1, D]]),
            )
            qres_sb = singles.tile([128, RPC // 128, D], f32, tag="qres_sb")
            nc.gpsimd.dma_start(
                out=qres_sb, in_=qres[:].rearrange("(mc p) d -> p mc d", p=128)
            )

            # projection outputs (feature-major, both heads stacked on partitions)
            qcb_sb = singles.tile([FPC, B * TQ], sdt, tag="qcb_sb")
            qpb_sb = singles.tile([FPC, B * TQ], sdt, tag="qpb_sb")
            kT_sb = singles.tile([FPC, B * TKV], sdt, tag="kT_sb")
            rT_sb = singles.tile([FPC, B * TKV], sdt, tag="rT_sb")
            # v in natural layout [kv-token partitions, chunk, head, 64+ones]
            v_sb = singles.tile(
                [128, (B * TKV) // 256, HPC, 2, 128], f8, tag="v_sb"
            )
            nc.vector.memset(v_sb, 0.0)
            nc.vector.memset(v_sb[:, :, :, :, DV], 1.0)
            ctx_sb = singles.tile([FPC, B * TQ], pdt, tag="ctx_sb")
            ones_bf = singles.tile([1, DV], bf16, tag="ones_bf")
            nc.vector.memset(ones_bf, 1.0)
            # row-selector for the 1/den broadcast matmul: column block r
            # of sel_bf picks the denominator row at partition 32r
            sel_bf = singles.tile([128, 4 * DV], bf16, tag="sel_bf")
            nc.vector.memset(sel_bf, 0.0)
            for r_ in range(4):
                nc.scalar.copy(
                    out=sel_bf[32 * r_ : 32 * r_ + 1, r_ * DV : (r_ + 1) * DV],
                    in_=ones_bf,
                )
            # softmax denominators at partitions 0/32/64/96, one row per
            # (pair-in-batch, half); unused partitions stay 1.0 so the
            # batch-wide reciprocal never produces inf/nan
            den_sb = singles.tile([128, B, 512], f32, tag="den_sb")
            nc.vector.memset(den_sb, 1.0)
            recip_sb = singles.tile([128, B, 512], bf16, tag="recip_sb")

            # identity (bf16) for PE-transposes
            ident_bf = singles.tile([128, 128], bf16, tag="ident_bf")
            from concourse.masks import make_identity

            make_identity(nc, ident_bf)

            # ========== Phases A+B interleaved: projections + attention ==========
            CH = 512  # token columns per projection step
            with contextlib.ExitStack() as phase_ab:
                pa_in = tc.alloc_tile_pool(name="pa_in", bufs=3)
                pa_ps = tc.alloc_tile_pool(name="pa_ps", bufs=4, space="PSUM")
                pa_psv = tc.alloc_tile_pool(name="pa_psv", bufs=4, space="PSUM")

                def emit_q_chunk(j):
                    q_in = pa_in.tile(
                        [128, D // 128, CH], f8, tag="xin", name=f"q_in{j}"
                    )
                    nc.sync.dma_start(
                        out=q_in,
                        in_=xqT[:].rearrange("(kc p) t -> p kc t", p=128)[
                            :, :, j * CH : (j + 1) * CH
                        ],
                    )
                    ps = pa_ps.tile([FPC, CH], f32, tag="ps", name=f"ps_q{j}")
                    for kc in range(D // 256):
                        nc.tensor.matmul(
                            ps,
                            wq_sb[:, 2 * kc : 2 * kc + 2, :],
                            q_in[:, 2 * kc : 2 * kc + 2, :],
                            start=(kc == 0),
                            stop=(kc == D // 256 - 1),
                            perf_mode=DR,
                        )
                    sl = slice(j * CH, (j + 1) * CH)
                    nc.vector.tensor_scalar_add(
                        out=qcb_sb[:, sl], in0=ps, scalar1=cb_sb
                    )
                    nc.vector.tensor_scalar_add(
                        out=qpb_sb[:, sl], in0=ps, scalar1=pb_sb
                    )

                def emit_kvr_chunk(j):
                    kv_in = pa_in.tile(
                        [128, D // 128, CH], f8, tag="xin", name=f"kv_in{j}"
                    )
                    nc.sync.dma_start(
                        out=kv_in,
                        in_=xkvT[:].rearrange("(kc p) t -> p kc t", p=128)[
                            :, :, j * CH : (j + 1) * CH
                        ],
                    )
                    ps = pa_ps.tile([FPC, CH], f32, tag="ps", name=f"ps_k{j}")
                    for kc in range(D // 256):
                        nc.tensor.matmul(
                            ps,
                            wk_sb[:, 2 * kc : 2 * kc + 2, :],
                            kv_in[:, 2 * kc : 2 * kc + 2, :],
                            start=(kc == 0),
                            stop=(kc == D // 256 - 1),
                            perf_mode=DR,
                        )
                    sl = slice(j * CH, (j + 1) * CH)
                    nc.vector.tensor_copy(out=kT_sb[:, sl], in_=ps)
                    # v: compute vT (feature-major, fast N) then PE-transpose
                    # into natural [tokens, feats] bf16 tiles
                    psvt = pa_ps.tile([FPC, CH], f32, tag="ps", name=f"psvt{j}")
                    for kc in range(D // 256):
                        nc.tensor.matmul(
                            psvt,
                            wv_sb[:, 2 * kc : 2 * kc + 2, :],
                            kv_in[:, 2 * kc : 2 * kc + 2, :],
                            start=(kc == 0),
                            stop=(kc == D // 256 - 1),
                            perf_mode=DR,
                        )
                    vt_t = pa_in.tile([FPC, CH], pdt, tag="vt_t", name=f"vt_t{j}")
                    nc.vector.tensor_copy(out=vt_t, in_=psvt)
                    for s in range(CH // 128):
                        psv = pa_psv.tile([128, FPC], pdt, tag="psv", name=f"psv{j}_{s}")
                        nc.tensor.transpose(
                            psv,
                            vt_t[:, s * 128 : (s + 1) * 128],
                            ident_bf,
                        )
                        cidx = j * (CH // 128) + s
                        for hh in range(HPC):
                            nc.scalar.copy(
                                out=v_sb[:, cidx // 2, hh, cidx % 2, 0:DV],
                                in_=psv[:, hh * DV : (hh + 1) * DV],
                            )
                    r_in = pa_in.tile(
                        [128, D // 128, CH], f8, tag="xin2", name=f"r_in{j}"
                    )
                    nc.scalar.dma_start(
                        out=r_in,
                        in_=xrT[:].rearrange("(kc p) t -> p kc t", p=128)[
                            :, :, j * CH : (j + 1) * CH
                        ],
                    )
                    ps2 = pa_ps.tile([FPC, CH], f32, tag="ps", name=f"ps_r{j}")
                    for kc in range(D // 256):
                        nc.tensor.matmul(
                            ps2,
                            wr_sb[:, 2 * kc : 2 * kc + 2, :],
                            r_in[:, 2 * kc : 2 * kc + 2, :],
                            start=(kc == 0),
                            stop=(kc == D // 256 - 1),
                            perf_mode=DR,
                        )
                    nc.vector.tensor_copy(out=rT_sb[:, sl], in_=ps2)

                for j in range(2):
                    emit_q_chunk(j)
                for j in range(4):
                    emit_kvr_chunk(j)
                for j in range(2, 4):
                    emit_q_chunk(j)
                for j in range(4, 8):
                    emit_kvr_chunk(j)
                pa_psv.release()
                pa_ps.release()
                pa_in.release()

                # attention pools
                pb_rows = tc.alloc_tile_pool(name="pb_rows", bufs=4)
                pb_bdt = tc.alloc_tile_pool(name="pb_bdt", bufs=4)
                pb_eac = tc.alloc_tile_pool(name="pb_eac", bufs=4)
                pb_prod = tc.alloc_tile_pool(name="pb_prod", bufs=4)
                pb_bc = tc.alloc_tile_pool(name="pb_bc", bufs=2)
                pb_small = tc.alloc_tile_pool(name="pb_small", bufs=2)
                pb_ps = tc.alloc_tile_pool(name="pb_ps", bufs=2, space="PSUM")
                pb_ps2 = tc.alloc_tile_pool(name="pb_ps2", bufs=2, space="PSUM")
                pb_ctx = tc.alloc_tile_pool(name="pb_ctx", bufs=2, space="PSUM")

                def bd_raw_stages(pi, b, hh, t):
                    # exp(bd/8) rows for q chunk t, q-major, unshifted.
                    # Returned as fine-grained thunks (one matmul+exp each,
                    # plus the DMA write) so they interleave with the score
                    # pipeline without bunching up on the scalar engine.
                    qf = slice(64 * hh, 64 * hh + 64)
                    n0 = 1 if t < 4 else 0
                    state = {}

                    def mk_mm(n):
                        def thunk():
                            if "row" not in state:
                                state["row"] = pb_rows.tile(
                                    [128, TKV],
                                    bf16,
                                    tag="bd_row",
                                    name=f"bd_row{pi}_{t}",
                                )
                            ps_bd = pb_ps.tile([128, 512], f32, tag="ps_sc")
                            nc.tensor.matmul(
                                ps_bd,
                                qpb_sb[
                                    qf, b * TQ + t * 128 : b * TQ + (t + 1) * 128
                                ],
                                rT_sb[
                                    qf, b * TKV + 512 * n : b * TKV + 512 * (n + 1)
                                ],
                                start=True,
                                stop=True,
                            )
                            nc.scalar.activation(
                                out=state["row"][:, 512 * n : 512 * (n + 1)],
                                in_=ps_bd,
                                func=Exp,
                                scale=0.125,
                            )

                        return thunk

                    def wr_thunk():
                        nc.gpsimd.dma_start(
                            out=AP(
                                tensor=bd_dram[pi][t // 4][:].tensor,
                                offset=(t % 4) * 128 * TKV + 512 * n0,
                                ap=[[TKV, 128], [1, TKV - 512 * n0]],
                            ),
                            in_=state["row"][:, 512 * n0 : TKV],
                        )

                    return [mk_mm(n) for n in range(n0, NK)] + [wr_thunk]

                def attn_half(pi, b, hh, h, fillers, pending_finish):
                    qf = slice(64 * hh, 64 * hh + 64)
                    kcmax = 12 + 4 * h
                    ps_ctx = pb_ctx.tile(
                        [128, 512], f32, tag="ps_ctx", name=f"psctx{pi}_{h}"
                    )
                    bd_tiles = {}
                    prod_tiles = {}

                    def issue_read(kcp):
                        # shifted+transposed exp(bd) tiles for kc pair
                        # (2kcp, 2kcp+1): [kv 128, 2, q 512] via one XBAR read;
                        # only rows covering the unmasked q range are read.
                        qlo = max(0, 128 * (2 * kcp - 8) - 512 * h)
                        ebdT = pb_bdt.tile([128, 2, 512], bf16, tag="ebdT")
                        nc.sync.dma_start(
                            out=ebdT[:, :, qlo:512],
                            in_=AP(
                                tensor=bd_dram[pi][h][:].tensor,
                                offset=(TQ - 1 - 512 * h)
                                + 256 * kcp
                                + qlo * (TKV - 1),
                                ap=[[TKV - 1, 512 - qlo], [1, 256]],
                            ),
                            transpose=True,
                        )
                        bd_tiles[kcp] = ebdT

                    def score_stage(kc):
                        eng = nc.vector
                        bdt = bd_tiles[kc // 2][:, kc % 2, :]
                        if kc >= 8 + 4 * h:
                            # keep where q >= k - R_OFF, i.e.
                            # j + (512h + R_OFF - 128 kc) - p >= 0
                            nc.gpsimd.affine_select(
                                out=bdt,
                                in_=bdt,
                                pattern=[[1, 512]],
                                compare_op=ALU.is_ge,
                                fill=0.0,
                                base=512 * h + R_OFF - 128 * kc,
                                channel_multiplier=-1,
                            )
                        ps_sc = pb_ps2.tile([128, 512], f32, tag="ps_sc2")
                        nc.tensor.matmul(
                            ps_sc,
                            kT_sb[qf, b * TKV + 128 * kc : b * TKV + 128 * (kc + 1)],
                            qcb_sb[qf, b * TQ + 512 * h : b * TQ + 512 * (h + 1)],
                            start=True,
                            stop=True,
                        )
                        eacT = pb_eac.tile([128, 512], bf16, tag="eacT")
                        nc.scalar.activation(
                            out=eacT, in_=ps_sc, func=Exp, scale=0.125
                        )
                        pairi = kc // 2
                        if kc % 2 == 0:
                            prod_tiles[pairi] = pb_prod.tile(
                                [128, 2, 512], f8, tag="expT",
                                name=f"expT{pi}_{h}_{pairi}",
                            )
                        eng.tensor_mul(
                            out=prod_tiles[pairi][:, kc % 2, :], in0=eacT, in1=bdt
                        )
                        if kc % 2 == 1:
                            bd_tiles.pop(kc // 2)

                    def ctx_pair(pairi):
                        nc.tensor.matmul(
                            ps_ctx,
                            v_sb[:, b * (TKV // 256) + pairi, hh, :, :],
                            prod_tiles.pop(pairi),
                            start=(pairi == 0),
                            stop=(pairi == kcmax // 2 - 1),
                            perf_mode=DR,
                        )

                    for k2 in range(2):
                        issue_read(k2)
                    for kc in range(kcmax):
                        if kc % 2 == 0 and kc // 2 + 2 < kcmax // 2:
                            issue_read(kc // 2 + 2)
                        score_stage(kc)
                        if kc % 2 == 1 and kc // 2 >= 1:
                            ctx_pair(kc // 2 - 1)
                        if kc == 1 and pending_finish is not None:
                            pending_finish()
                            pending_finish = None
                        # spread filler stages so the scalar engine never
                        # queues bd-exp work ahead of the critical eacT exp
                        slots_left = kcmax - kc
                        take = (len(fillers) + slots_left - 1) // slots_left
                        for _ in range(min(take, len(fillers))):
                            fillers.pop(0)()
                    ctx_pair(kcmax // 2 - 1)
                    if pending_finish is not None:
                        pending_finish()

                    def finish():
                        # stash denominator row + unnormalized ctx (bf16);
                        # the actual 1/den normalize happens per batch.
                        ridx = 2 * (pi % 2) + h
                        nc.scalar.copy(
                            out=den_sb[32 * ridx : 32 * ridx + 1, b, :],
                            in_=ps_ctx[DV : DV + 1, :],
                        )
                        nc.vector.tensor_copy(
                            out=ctx_sb[
                                qf, b * TQ + 512 * h : b * TQ + 512 * (h + 1)
                            ],
                            in_=ps_ctx[0:DV, :],
                        )

                    return finish

                def normalize_batch(b):
                    with nc.allow_low_precision(
                        reason="bf16 1/denominator matches baseline attn bf16"
                    ):
                        nc.vector.reciprocal(
                            recip_sb[:, b, :], den_sb[:, b, :]
                        )
                    for ridx in range(4):
                        hh = ridx // 2
                        h = ridx % 2
                        qf = slice(64 * hh, 64 * hh + 64)
                        cols = slice(b * TQ + 512 * h, b * TQ + 512 * (h + 1))
                        ps_b = pb_ctx.tile(
                            [DV, 512], f32, tag="ps_ctx", name=f"ps_b{b}_{ridx}"
                        )
                        nc.tensor.matmul(
                            ps_b,
                            sel_bf[:, ridx * DV : (ridx + 1) * DV],
                            recip_sb[:, b, :],
                            start=True,
                            stop=True,
                        )
                        bcast = pb_bc.tile(
                            [128, 512], bf16, tag="bcast", name=f"bc{b}_{ridx}"
                        )
                        nc.scalar.copy(out=bcast[qf, :], in_=ps_b)
                        nc.vector.tensor_mul(
                            out=ctx_sb[qf, cols],
                            in0=ctx_sb[qf, cols],
                            in1=bcast[qf, :],
                        )

                def exchange_batch(b):
                    # ship this batch's ctx^T; chunk j (128 q cols) -> core j
                    nc.sync.dma_start(
                        out=a2a_in[b][:].rearrange("(j p) t -> p j t", p=FPC),
                        in_=ctx_sb[:, b * TQ : (b + 1) * TQ].rearrange(
                            "p (j t) -> p j t", t=TQ // 8
                        ),
                    )
                    if n_cores > 1:
                        nc.gpsimd.collective_compute(
                            "AllToAll",
                            ALU.bypass,
                            replica_groups=[list(range(n_cores))],
                            ins=[a2a_in[b][:]],
                            outs=[a2a_out[b][:]],
                        )
                    else:
                        # single-core profiling variant: plain copy instead
                        nc.sync.dma_start(out=a2a_out[b][:], in_=a2a_in[b][:])

                pc = tc.alloc_tile_pool(name="pc", bufs=3)
                pc_ps = tc.alloc_tile_pool(name="pc_ps", bufs=2, space="PSUM")
                pc_small = tc.alloc_tile_pool(name="pc_small", bufs=4)

                def phase_c_mc(mc):
                    # output projection + residual + LayerNorm for this
                    # core's 128 q rows of batch mc
                    ps_o = [
                        pc_ps.tile([128, 512], f32, tag="ps_o",
                                   name=f"ps_o{mc}_{nn_}")
                        for nn_ in range(2)
                    ]
                    for kc in range(D // 128):
                        lhs = pc.tile([128, 128], pdt, tag="octx")
                        nc.sync.dma_start(
                            out=lhs,
                            in_=a2a_out[mc][kc * 128 : (kc + 1) * 128, :],
                        )
                        for nn in range(2):
                            nc.tensor.matmul(
                                ps_o[nn],
                                lhs,
                                wo_sb[:, kc, nn * 512 : (nn + 1) * 512],
                                start=(kc == 0),
                                stop=(kc == D // 128 - 1),
                            )
                    o_sb = pc.tile([128, D], f32, tag="o_sb")
                    for nn in range(2):
                        nc.vector.tensor_add(
                            out=o_sb[:, nn * 512 : (nn + 1) * 512],
                            in0=ps_o[nn],
                            in1=qres_sb[:, mc, nn * 512 : (nn + 1) * 512],
                        )
                    # LayerNorm over the free (feature) dim
                    stats = pc_small.tile([128, 2, 6], f32, tag="stats")
                    for sg in range(2):
                        nc.vector.bn_stats(
                            out=stats[:, sg, :], in_=o_sb[:, sg * 512 : (sg + 1) * 512]
                        )
                    mv = pc_small.tile([128, 2], f32, tag="mv")
                    nc.vector.bn_aggr(out=mv, in_=stats)
                    mean, var = mv[:, 0:1], mv[:, 1:2]
                    xve = pc_small.tile([128, 1], f32, tag="xve")
                    nc.vector.tensor_scalar_add(out=xve, in0=var, scalar1=eps_sb)
                    std = pc_small.tile([128, 1], f32, tag="std")
                    nc.scalar.activation(out=std, in_=var, func=Sqrt, bias=eps_sb)
                    rstd = pc_small.tile([128, 1], f32, tag="rstd")
                    nc.vector.reciprocal(rstd, std)
                    # one Newton step for rsqrt accuracy:
                    # r <- r * (1.5 - 0.5 * x * r^2)
                    tnw = pc_small.tile([128, 1], f32, tag="tnw")
                    nc.vector.tensor_mul(out=tnw, in0=rstd, in1=rstd)
                    nc.vector.tensor_mul(out=tnw, in0=tnw, in1=xve)
                    nc.vector.tensor_scalar(
                        out=tnw, in0=tnw, scalar1=-0.5, scalar2=1.5,
                        op0=ALU.mult, op1=ALU.add,
                    )
                    nc.vector.tensor_scalar_mul(out=rstd, in0=rstd, scalar1=tnw)
                    nc.vector.tensor_scalar(
                        out=o_sb, in0=o_sb, scalar1=mean, scalar2=rstd,
                        op0=ALU.subtract, op1=ALU.mult,
                    )
                    nc.vector.tensor_mul(out=o_sb, in0=o_sb, in1=gamma_sb)
                    nc.vector.tensor_add(out=o_sb, in0=o_sb, in1=beta_sb)
                    nc.sync.dma_start(
                        out=out[mc * 128 : (mc + 1) * 128, :], in_=o_sb
                    )

                pairs = [(0, 0, 0), (1, 0, 1), (2, 1, 0), (3, 1, 1)]
                pending = None
                for idx, (pi, b, hh) in enumerate(pairs):
                    if idx == 0:
                        for t in range(4):
                            for th in bd_raw_stages(pi, b, hh, t):
                                th()
                    if idx == 3:
                        # batch-0 output projection overlaps batch-1 attention
                        phase_c_mc(0)
                    fill0 = []
                    for t in range(4):
                        fill0 += bd_raw_stages(pi, b, hh, 4 + t)
                    pending = attn_half(pi, b, hh, 0, fill0, pending)
                    fill1 = []
                    if idx + 1 < 4:
                        pj, bj, hj = pairs[idx + 1]
                        for t in range(4):
                            fill1 += bd_raw_stages(pj, bj, hj, t)
                    pending = attn_half(pi, b, hh, 1, fill1, pending)
                    if idx == 1 or idx == 3:
                        pending()
                        pending = None
                        normalize_batch(b)
                        exchange_batch(b)

                phase_c_mc(1)

                pc_small.release()
                pc_ps.release()
                pc.release()
                pb_ctx.release()
                pb_ps2.release()
                pb_ps.release()
                pb_small.release()
                pb_bc.release()
                pb_prod.release()
                pb_eac.release()
                pb_bdt.release()
                pb_rows.release()
    return nc


def _make_in_maps(inputs, mm_dtype="bfloat16"):
    query = np.asarray(inputs["query"], np.float32)
    key_value = np.asarray(inputs["key_value"], np.float32)
    relative = np.asarray(inputs["relative"], np.float32)
    content_bias = np.asarray(inputs["content_bias"], np.float32)
    position_bias = np.asarray(inputs["position_bias"], np.float32)
    Wq, Wk = np.asarray(inputs["Wq"], np.float32), np.asarray(inputs["Wk"], np.float32)
    Wv, Wr = np.asarray(inputs["Wv"], np.float32), np.asarray(inputs["Wr"], np.float32)
    Wo = np.ascontiguousarray(np.asarray(inputs["Wo"], np.float32))
    ln_gamma = np.asarray(inputs["ln_gamma"], np.float32)
    ln_beta = np.asarray(inputs["ln_beta"], np.float32)

    qflat = query.reshape(B * TQ, D)
    import ml_dtypes

    mdt = ml_dtypes.bfloat16
    f8dt = ml_dtypes.float8_e4m3fn
    xqT = np.ascontiguousarray(qflat.T).astype(f8dt)
    xkvT = np.ascontiguousarray(key_value.reshape(B * TKV, D).T).astype(f8dt)
    xrT = np.ascontiguousarray(relative.reshape(B * TKV, D).T).astype(f8dt)
    Wq, Wk = Wq.astype(f8dt), Wk.astype(f8dt)
    Wv, Wr = Wv.astype(f8dt), Wr.astype(f8dt)
    Wo = Wo.astype(mdt)
    cb = content_bias.reshape(NH, DV)
    pb = position_bias.reshape(NH, DV)

    in_maps = []
    for c in range(N_CORES):
        fs = slice(FPC * c, FPC * (c + 1))
        in_maps.append(
            {
                "xqT": xqT,
                "xkvT": xkvT,
                "xrT": xrT,
                "wq": np.ascontiguousarray(Wq[:, fs]),
                "wk": np.ascontiguousarray(Wk[:, fs]),
                "wv": np.ascontiguousarray(Wv[:, fs]),
                "wr": np.ascontiguousarray(Wr[:, fs]),
                "wo": Wo,
                "cbv": np.ascontiguousarray(
                    cb[HPC * c : HPC * (c + 1)].reshape(FPC, 1)
                ),
                "pbv": np.ascontiguousarray(
                    pb[HPC * c : HPC * (c + 1)].reshape(FPC, 1)
                ),
                "qres": np.ascontiguousarray(
                    np.concatenate(
                        [
                            qflat[128 * c : 128 * (c + 1)],
                            qflat[TQ + 128 * c : TQ + 128 * (c + 1)],
                        ]
                    )
                ),
                "gamma": ln_gamma,
                "beta": ln_beta,
            }
        )
    return in_maps


def run_on_hw(inputs, trace=False, score_dtype="bfloat16", proj_dtype="bfloat16"):
    from concourse.bass_utils import run_bass_kernel_spmd

    key = (score_dtype, proj_dtype)
    nc = _CACHE.get(key)
    if nc is None:
        nc = build_program(score_dtype=score_dtype, proj_dtype=proj_dtype)
        _CACHE[key] = nc
    in_maps = _make_in_maps(inputs, mm_dtype=proj_dtype)
    res = run_bass_kernel_spmd(nc, in_maps, list(range(N_CORES)), trace=trace)
    full = np.empty((B * TQ, D), np.float32)
    for c in range(N_CORES):
        o = np.asarray(res.results[c]["out"])
        full[128 * c : 128 * (c + 1)] = o[:128]
        full[TQ + 128 * c : TQ + 128 * (c + 1)] = o[128:]
    return full.reshape(B, TQ, D), res


def kernel(**inputs) -> np.ndarray:
    out, _ = run_on_hw(inputs)
    return out


# revision 50
# speedup vs baseline: 1.0061x; 1.0061x over previous
"""Trainium2 Bass kernel for Transformer-XL style relative-position attention.

Reference computation (B=2, Tq=1024, Tkv=2048, D=1024, H=16, Dv=64):
    q/k/v/r projections, ac = (q+cb)@k^T, bd = rel_shift((q+pb)@r^T),
    softmax((ac+bd)/8) with causal-with-memory mask, ctx = attn@v,
    out = LN(ctx@Wo + query).

Sharding (Megatron-style tensor parallel over heads, 8 cores):
  - each core owns 2 heads: Wq/Wk/Wv/Wr column shards [1024,128], biases
    shard with heads.
  - activations (transposed on host to feature-major) are broadcast.
  - after per-head attention, ctx^T shards are exchanged with a single
    AllToAll so that each core ends up with the full ctx^T for 1/8 of the
    token rows; each core then does that row-slice of ctx@Wo + residual +
    LayerNorm with the full Wo.

Device-side structure (transposed-scores design):
  - scores are computed TRANSPOSED (kv on partitions, q on the free dim):
    acT tiles come straight from a matmul with kT as the stationary
    operand; this makes the softmax output directly consumable by the
    attn@v contraction with NO attention-matrix transpose or DRAM
    round-trip.
  - rel_shift: bd is computed q-major (dense PE work), EXPONENTIATED
    (exp commutes with the shift), written bf16 to a flat DRAM scratch,
    and read back through a strided AP with row stride Tkv-1 PLUS
    transpose=True (hardware XBAR transpose), which lands the *shifted,
    transposed* exp(bd/8) tiles in SBUF in one step.
  - softmax numerator: exp((ac+bd)/8) = exp(ac/8) * exp(bd/8): the
    scalar engine exponentiates acT from PSUM, and the product runs on
    gpsimd/vector (SBUF-only operands), so no engine ever needs an
    (ac+bd) add against PSUM.
  - causal mask applied to the exp(bd) tiles with affine_select fill=0.
  - softmax denominators come for free from a ones-column appended to v:
    the attn@v matmul accumulates sum(exp) in psum row 64.
  - 1/denominator (per q) is broadcast across the 64 feature partitions
    with a rank-1 matmul (ones ⊗ recip) and applied to the small ctx^T
    tile instead of the big attention matrix.
"""

import numpy as np

# problem shapes (hardcoded per contract)
B, TQ, TKV, D, NH, DV = 2, 1024, 2048, 1024, 16, 64
N_CORES = 8
HPC = NH // N_CORES          # heads per core = 2
FPC = HPC * DV               # head-feature columns per core = 128
RPC = (B * TQ) // N_CORES    # output token rows per core = 256
R_OFF = TKV - TQ             # causal memory offset = 1024
LN_EPS = 1e-5
NT = TQ // 128               # query row chunks = 8
NK = TKV // 512              # key col chunks of 512 = 4

_CACHE = {}


def _patched_tc_class():
    """TileContext whose kernel-tail drain splits sem waits one per drain.

    The walrus build in this container rejects CTRL-type instructions
    (InstDrain) carrying more than one sync-wait command.
    """
    import concourse.mybir as mybir
    import concourse.tile as tile
    from concourse.vector_clock import ScopedClock

    class TC(tile.TileContext):
        def _commit_instruction(self, inst, lazy_reg_writes=True):
            # This walrus build rejects instructions carrying more than one
            # sync-wait command; hoist extras onto preceding NoOp carriers.
            si = getattr(inst, "sync_info", None)
            if (
                si is not None
                and si.on_wait
                and len(si.on_wait) > 1
                and inst.engine != mybir.EngineType.Unassigned
            ):
                waits = list(si.on_wait)
                inst.sync_info = mybir.SyncInfo(
                    on_wait=[waits[-1]], on_update=list(si.on_update or [])
                )
                for w in waits[:-1]:
                    ev = mybir.InstNoOp(
                        name=f"I-wsplit-{self.nc.next_id()}", ins=[], outs=[]
                    )
                    ev.engine = inst.engine
                    ev.sync_info = mybir.SyncInfo(on_wait=[w], on_update=[])
                    self._add_instruction(ev)
            return super()._commit_instruction(inst, lazy_reg_writes)

        def _drain_and_barrier(self, tick_clock, wait_clock):
            nc = self.nc
            drain_inst = nc.sync.drain()
            wait_clock.add_sem_waits(
                drain_inst.ins, ScopedClock({None: tick_clock.global_clock})
            )
            inner = drain_inst.ins
            si = inner.sync_info
            waits = list(si.on_wait) if si and si.on_wait else []
            if len(waits) > 1:
                inner.sync_info = mybir.SyncInfo(
                    on_wait=waits[:1], on_update=list(si.on_update or [])
                )
                for w in waits[1:]:
                    d2 = nc.sync.drain()
                    d2.ins.sync_info = mybir.SyncInfo(on_wait=[w], on_update=[])
            nc.all_engine_barrier()
            assert self.sems is not None
            popped = nc._tile_sem_poison_stack.pop()
            assert popped is self._sem_poison
            nc.clear_and_free_semaphores(list(self.sems.allocated().values()))
            nc.all_engine_barrier()

    return TC


def build_program(score_dtype="bfloat16", proj_dtype="bfloat16", n_cores=N_CORES):
    """Build the SPMD Bass program (identical on all 8 cores).

    n_cores=1 builds a single-core variant (collective replaced by a
    self-copy) for profiling; its output is only valid for core 0's
    feature shard.
    """
    import concourse.bass as bass
    import concourse.mybir as mybir
    from concourse.bass import AP

    f32 = mybir.dt.float32
    bf16 = mybir.dt.bfloat16
    f8 = mybir.dt.float8e4
    DR = mybir.MatmulPerfMode.DoubleRow
    pdt = bf16
    sdt = bf16
    TC = _patched_tc_class()

    nc = bass.Bass()

    # ---- I/O ----
    xqT = nc.dram_tensor("xqT", [D, B * TQ], f8, kind="ExternalInput")
    xkvT = nc.dram_tensor("xkvT", [D, B * TKV], f8, kind="ExternalInput")
    xrT = nc.dram_tensor("xrT", [D, B * TKV], f8, kind="ExternalInput")
    wq = nc.dram_tensor("wq", [D, FPC], f8, kind="ExternalInput")
    wk = nc.dram_tensor("wk", [D, FPC], f8, kind="ExternalInput")
    wv = nc.dram_tensor("wv", [D, FPC], f8, kind="ExternalInput")
    wr = nc.dram_tensor("wr", [D, FPC], f8, kind="ExternalInput")
    wo = nc.dram_tensor("wo", [D, D], pdt, kind="ExternalInput")
    cbv = nc.dram_tensor("cbv", [FPC, 1], f32, kind="ExternalInput")
    pbv = nc.dram_tensor("pbv", [FPC, 1], f32, kind="ExternalInput")
    qres = nc.dram_tensor("qres", [RPC, D], f32, kind="ExternalInput")
    gamma = nc.dram_tensor("gamma", [D], f32, kind="ExternalInput")
    beta = nc.dram_tensor("beta", [D], f32, kind="ExternalInput")
    out = nc.dram_tensor("out", [RPC, D], f32, kind="ExternalOutput")

    # ---- internal DRAM scratch ----
    # raw bd per (pair, q-half): flat [512 rows x TKV]; the shifted,
    # transposed read only ever depends on its own half's rows.
    bd_dram = [
        [nc.dram_tensor(f"bd_dram{p}_{h}", [512 * TKV], bf16) for h in range(2)]
        for p in range(4)
    ]
    # one AllToAll per batch: core c owns q rows [128c, 128c+128) of EACH
    # batch, so batch-0 exchange + output projection overlap batch-1 attention
    a2a_in = [nc.dram_tensor(f"a2a_in{b}", [N_CORES * FPC, TQ // 8], pdt) for b in range(B)]
    a2a_out = [nc.dram_tensor(f"a2a_out{b}", [N_CORES * FPC, TQ // 8], pdt) for b in range(B)]

    Exp = mybir.ActivationFunctionType.Exp
    Identity = mybir.ActivationFunctionType.Identity
    Sqrt = mybir.ActivationFunctionType.Sqrt
    ALU = mybir.AluOpType

    with TC(nc) as tc:
        import contextlib

        with contextlib.ExitStack() as ctx:
            singles = ctx.enter_context(tc.tile_pool(name="singles", bufs=1))

            # ---- static SBUF tensors ----
            wq_sb = singles.tile([128, D // 128, FPC], f8, tag="wq_sb")
            wk_sb = singles.tile([128, D // 128, FPC], f8, tag="wk_sb")
            wv_sb = singles.tile([128, D // 128, FPC], f8, tag="wv_sb")
            wr_sb = singles.tile([128, D // 128, FPC], f8, tag="wr_sb")
            for w_sb, w_dr in ((wq_sb, wq), (wk_sb, wk), (wv_sb, wv), (wr_sb, wr)):
                nc.gpsimd.dma_start(
                    out=w_sb, in_=w_dr[:].rearrange("(kc p) f -> p kc f", p=128)
                )
            wo_sb = singles.tile([128, D // 128, D], pdt, tag="wo_sb")
            nc.gpsimd.dma_start(
                out=wo_sb, in_=wo[:].rearrange("(kc p) d -> p kc d", p=128)
            )
            cb_sb = singles.tile([FPC, 1], f32, tag="cb_sb")
            pb_sb = singles.tile([FPC, 1], f32, tag="pb_sb")
            nc.sync.dma_start(out=cb_sb, in_=cbv[:])
            nc.sync.dma_start(out=pb_sb, in_=pbv[:])
            eps_sb = singles.tile([128, 1], f32, tag="eps_sb")
            nc.vector.memset(eps_sb, LN_EPS)
            gamma_sb = singles.tile([128, D], f32, tag="gamma_sb")
            beta_sb = singles.tile([128, D], f32, tag="beta_sb")
            nc.gpsimd.dma_start(
                out=gamma_sb,
                in_=AP(tensor=gamma[:].tensor, offset=0, ap=[[0, 128], [1, D]]),
            )
            nc.gpsimd.dma_start(
                out=beta_sb,
                in_=AP(tensor=beta[:].tensor, offset=0, ap=[[0, 128], [1, D]]),
            )
            qres_sb = singles.tile([128, RPC // 128, D], f32, tag="qres_sb")
            nc.gpsimd.dma_start(
                out=qres_sb, in_=qres[:].rearrange("(mc p) d -> p mc d", p=128)
            )

            # projection outputs (feature-major, both heads stacked on partitions)
            qcb_sb = singles.tile([FPC, B * TQ], sdt, tag="qcb_sb")
            qpb_sb = singles.tile([FPC, B * TQ], sdt, tag="qpb_sb")
            kT_sb = singles.tile([FPC, B * TKV], sdt, tag="kT_sb")
            rT_sb = singles.tile([FPC, B * TKV], sdt, tag="rT_sb")
            # v in natural layout [kv-token partitions, chunk, head, 64+ones]
            v_sb = singles.tile(
                [128, (B * TKV) // 256, HPC, 2, 128], f8, tag="v_sb"
            )
            nc.vector.memset(v_sb, 0.0)
            nc.vector.memset(v_sb[:, :, :, :, DV], 1.0)
            ctx_sb = singles.tile([FPC, B * TQ], pdt, tag="ctx_sb")
            ones_bf = singles.tile([1, DV], bf16, tag="ones_bf")
            nc.vector.memset(ones_bf, 1.0)
            # row-selector for the 1/den broadcast matmul: column block r
            # of sel_bf picks the denominator row at partition 32r
            sel_bf = singles.tile([128, 4 * DV], bf16, tag="sel_bf")
            nc.vector.memset(sel_bf, 0.0)
            for r_ in range(4):
                nc.scalar.copy(
                    out=sel_bf[32 * r_ : 32 * r_ + 1, r_ * DV : (r_ + 1) * DV],
                    in_=ones_bf,
                )
            # softmax denominators at partitions 0/32/64/96, one row per
            # (pair-in-batch, half); unused partitions stay 1.0 so the
            # batch-wide reciprocal never produces inf/nan
            den_sb = singles.tile([128, B, 512], f32, tag="den_sb")
            nc.vector.memset(den_sb, 1.0)
            recip_sb = singles.tile([128, B, 512], bf16, tag="recip_sb")

            # identity (bf16) for PE-transposes
            ident_bf = singles.tile([128, 128], bf16, tag="ident_bf")
            from concourse.masks import make_identity

            make_identity(nc, ident_bf)

            # ========== Phases A+B interleaved: projections + attention ==========
            CH = 512  # token columns per projection step
            with contextlib.ExitStack() as phase_ab:
                pa_in = tc.alloc_tile_pool(name="pa_in", bufs=2)
                pa_ps = tc.alloc_tile_pool(name="pa_ps", bufs=4, space="PSUM")
                pa_psv = tc.alloc_tile_pool(name="pa_psv", bufs=4, space="PSUM")

                def emit_q_chunk(j):
                    q_in = pa_in.tile(
                        [128, D // 128, CH], f8, tag="xin", name=f"q_in{j}"
                    )
                    nc.sync.dma_start(
                        out=q_in,
                        in_=xqT[:].rearrange("(kc p) t -> p kc t", p=128)[
                            :, :, j * CH : (j + 1) * CH
                        ],
                    )
                    ps = pa_ps.tile([FPC, CH], f32, tag="ps", name=f"ps_q{j}")
                    for kc in range(D // 256):
                        nc.tensor.matmul(
                            ps,
                            wq_sb[:, 2 * kc : 2 * kc + 2, :],
                            q_in[:, 2 * kc : 2 * kc + 2, :],
                            start=(kc == 0),
                            stop=(kc == D // 256 - 1),
                            perf_mode=DR,
                        )
                    sl = slice(j * CH, (j + 1) * CH)
                    nc.vector.tensor_scalar_add(
                        out=qcb_sb[:, sl], in0=ps, scalar1=cb_sb
                    )
                    nc.vector.tensor_scalar_add(
                        out=qpb_sb[:, sl], in0=ps, scalar1=pb_sb
                    )

                def emit_kvr_chunk(j):
                    kv_in = pa_in.tile(
                        [128, D // 128, CH], f8, tag="xin", name=f"kv_in{j}"
                    )
                    nc.sync.dma_start(
                        out=kv_in,
                        in_=xkvT[:].rearrange("(kc p) t -> p kc t", p=128)[
                            :, :, j * CH : (j + 1) * CH
                        ],
                    )
                    ps = pa_ps.tile([FPC, CH], f32, tag="ps", name=f"ps_k{j}")
                    for kc in range(D // 256):
                        nc.tensor.matmul(
                            ps,
                            wk_sb[:, 2 * kc : 2 * kc + 2, :],
                            kv_in[:, 2 * kc : 2 * kc + 2, :],
                            start=(kc == 0),
                            stop=(kc == D // 256 - 1),
                            perf_mode=DR,
                        )
                    sl = slice(j * CH, (j + 1) * CH)
                    nc.vector.tensor_copy(out=kT_sb[:, sl], in_=ps)
                    # v: compute vT (feature-major, fast N) then PE-transpose
                    # into natural [tokens, feats] bf16 tiles
                    psvt = pa_ps.tile([FPC, CH], f32, tag="ps", name=f"psvt{j}")
                    for kc in range(D // 256):
                        nc.tensor.matmul(
                            psvt,
                            wv_sb[:, 2 * kc : 2 * kc + 2, :],
                            kv_in[:, 2 * kc : 2 * kc + 2, :],
                            start=(kc == 0),
                            stop=(kc == D // 256 - 1),
                            perf_mode=DR,
                        )
                    vt_t = pa_in.tile([FPC, CH], pdt, tag="vt_t", name=f"vt_t{j}")
                    nc.vector.tensor_copy(out=vt_t, in_=psvt)
                    for s in range(CH // 128):
                        psv = pa_psv.tile([128, FPC], pdt, tag="psv", name=f"psv{j}_{s}")
                        nc.tensor.transpose(
                            psv,
                            vt_t[:, s * 128 : (s + 1) * 128],
                            ident_bf,
                        )
                        cidx = j * (CH // 128) + s
                        for hh in range(HPC):
                            nc.scalar.copy(
                                out=v_sb[:, cidx // 2, hh, cidx % 2, 0:DV],
                                in_=psv[:, hh * DV : (hh + 1) * DV],
                            )
                    r_in = pa_in.tile(
                        [128, D // 128, CH], f8, tag="xin2", name=f"r_in{j}"
                    )
                    nc.scalar.dma_start(
                        out=r_in,
                        in_=xrT[:].rearrange("(kc p) t -> p kc t", p=128)[
                            :, :, j * CH : (j + 1) * CH
                        ],
                    )
                    ps2 = pa_ps.tile([FPC, CH], f32, tag="ps", name=f"ps_r{j}")
                    for kc in range(D // 256):
                        nc.tensor.matmul(
                            ps2,
                            wr_sb[:, 2 * kc : 2 * kc + 2, :],
                            r_in[:, 2 * kc : 2 * kc + 2, :],
                            start=(kc == 0),
                            stop=(kc == D // 256 - 1),
                            perf_mode=DR,
                        )
                    nc.vector.tensor_copy(out=rT_sb[:, sl], in_=ps2)

                for j in range(2):
                    emit_q_chunk(j)
                for j in range(4):
                    emit_kvr_chunk(j)
                for j in range(2, 4):
                    emit_q_chunk(j)
                for j in range(4, 8):
                    emit_kvr_chunk(j)
                pa_psv.release()
                pa_ps.release()
                pa_in.release()

                # attention pools
                pb_rows = tc.alloc_tile_pool(name="pb_rows", bufs=4)
                pb_bdt = tc.alloc_tile_pool(name="pb_bdt", bufs=4)
                pb_eac = tc.alloc_tile_pool(name="pb_eac", bufs=4)
                pb_prod = tc.alloc_tile_pool(name="pb_prod", bufs=4)
                pb_bc = tc.alloc_tile_pool(name="pb_bc", bufs=2)
                pb_small = tc.alloc_tile_pool(name="pb_small", bufs=2)
                pb_ps = tc.alloc_tile_pool(name="pb_ps", bufs=2, space="PSUM")
                pb_ps2 = tc.alloc_tile_pool(name="pb_ps2", bufs=2, space="PSUM")
                pb_ctx = tc.alloc_tile_pool(name="pb_ctx", bufs=2, space="PSUM")

                def bd_raw_stages(pi, b, hh, t):
                    # exp(bd/8) rows for q chunk t, q-major, unshifted.
                    # Returned as fine-grained thunks (one matmul+exp each,
                    # plus the DMA write) so they interleave with the score
                    # pipeline without bunching up on the scalar engine.
                    qf = slice(64 * hh, 64 * hh + 64)
                    n0 = 1 if t < 4 else 0
                    state = {}

                    def mk_mm(n):
                        def thunk():
                            if "row" not in state:
                                state["row"] = pb_rows.tile(
                                    [128, TKV],
                                    bf16,
                                    tag="bd_row",
                                    name=f"bd_row{pi}_{t}",
                                )
                            ps_bd = pb_ps.tile([128, 512], f32, tag="ps_sc")
                            nc.tensor.matmul(
                                ps_bd,
                                qpb_sb[
                                    qf, b * TQ + t * 128 : b * TQ + (t + 1) * 128
                                ],
                                rT_sb[
                                    qf, b * TKV + 512 * n : b * TKV + 512 * (n + 1)
                                ],
                                start=True,
                                stop=True,
                            )
                            nc.scalar.activation(
                                out=state["row"][:, 512 * n : 512 * (n + 1)],
                                in_=ps_bd,
                                func=Exp,
                                scale=0.125,
                            )

                        return thunk

                    def wr_thunk():
                        nc.gpsimd.dma_start(
                            out=AP(
                                tensor=bd_dram[pi][t // 4][:].tensor,
                                offset=(t % 4) * 128 * TKV + 512 * n0,
                                ap=[[TKV, 128], [1, TKV - 512 * n0]],
                            ),
                            in_=state["row"][:, 512 * n0 : TKV],
                        )

                    return [mk_mm(n) for n in range(n0, NK)] + [wr_thunk]

                def attn_half(pi, b, hh, h, fillers, pending_finish):
                    qf = slice(64 * hh, 64 * hh + 64)
                    kcmax = 12 + 4 * h
                    ps_ctx = pb_ctx.tile(
                        [128, 512], f32, tag="ps_ctx", name=f"psctx{pi}_{h}"
                    )
                    bd_tiles = {}
                    prod_tiles = {}

                    def issue_read(kcp):
                        # shifted+transposed exp(bd) tiles for kc pair
                        # (2kcp, 2kcp+1): [kv 128, 2, q 512] via one XBAR read;
                        # only rows covering the unmasked q range are read.
                        qlo = max(0, 128 * (2 * kcp - 8) - 512 * h)
                        ebdT = pb_bdt.tile([128, 2, 512], bf16, tag="ebdT")
                        nc.sync.dma_start(
                            out=ebdT[:, :, qlo:512],
                            in_=AP(
                                tensor=bd_dram[pi][h][:].tensor,
                                offset=(TQ - 1 - 512 * h)
                                + 256 * kcp
                                + qlo * (TKV - 1),
                                ap=[[TKV - 1, 512 - qlo], [1, 256]],
                            ),
                            transpose=True,
                        )
                        bd_tiles[kcp] = ebdT

                    def score_stage(kc):
                        eng = nc.vector
                        bdt = bd_tiles[kc // 2][:, kc % 2, :]
                        if kc >= 8 + 4 * h:
                            # keep where q >= k - R_OFF, i.e.
                            # j + (512h + R_OFF - 128 kc) - p >= 0
                            nc.gpsimd.affine_select(
                                out=bdt,
                                in_=bdt,
                                pattern=[[1, 512]],
                                compare_op=ALU.is_ge,
                                fill=0.0,
                                base=512 * h + R_OFF - 128 * kc,
                                channel_multiplier=-1,
                            )
                        ps_sc = pb_ps2.tile([128, 512], f32, tag="ps_sc2")
                        nc.tensor.matmul(
                            ps_sc,
                            kT_sb[qf, b * TKV + 128 * kc : b * TKV + 128 * (kc + 1)],
                            qcb_sb[qf, b * TQ + 512 * h : b * TQ + 512 * (h + 1)],
                            start=True,
                            stop=True,
                        )
                        eacT = pb_eac.tile([128, 512], bf16, tag="eacT")
                        nc.scalar.activation(
                            out=eacT, in_=ps_sc, func=Exp, scale=0.125
                        )
                        pairi = kc // 2
                        if kc % 2 == 0:
                            prod_tiles[pairi] = pb_prod.tile(
                                [128, 2, 512], f8, tag="expT",
                                name=f"expT{pi}_{h}_{pairi}",
                            )
                        eng.tensor_mul(
                            out=prod_tiles[pairi][:, kc % 2, :], in0=eacT, in1=bdt
                        )
                        if kc % 2 == 1:
                            bd_tiles.pop(kc // 2)

                    def ctx_pair(pairi):
                        nc.tensor.matmul(
                            ps_ctx,
                            v_sb[:, b * (TKV // 256) + pairi, hh, :, :],
                            prod_tiles.pop(pairi),
                            start=(pairi == 0),
                            stop=(pairi == kcmax // 2 - 1),
                            perf_mode=DR,
                        )

                    for k2 in range(2):
                        issue_read(k2)
                    for kc in range(kcmax):
                        if kc % 2 == 0 and kc // 2 + 2 < kcmax // 2:
                            issue_read(kc // 2 + 2)
                        score_stage(kc)
                        if kc % 2 == 1 and kc // 2 >= 1:
                            ctx_pair(kc // 2 - 1)
                        if kc == 1 and pending_finish is not None:
                            pending_finish()
                            pending_finish = None
                        # spread filler stages so the scalar engine never
                        # queues bd-exp work ahead of the critical eacT exp
                        slots_left = kcmax - kc
                        take = (len(fillers) + slots_left - 1) // slots_left
                        for _ in range(min(take, len(fillers))):
                            fillers.pop(0)()
                    ctx_pair(kcmax // 2 - 1)
                    if pending_finish is not None:
                        pending_finish()

                    def finish():
                        # stash denominator row + unnormalized ctx (bf16);
                        # the actual 1/den normalize happens per batch.
                        ridx = 2 * (pi % 2) + h
                        nc.scalar.copy(
                            out=den_sb[32 * ridx : 32 * ridx + 1, b, :],
                            in_=ps_ctx[DV : DV + 1, :],
                        )
                        nc.vector.tensor_copy(
                            out=ctx_sb[
                                qf, b * TQ + 512 * h : b * TQ + 512 * (h + 1)
                            ],
                            in_=ps_ctx[0:DV, :],
                        )

                    return finish

                def normalize_batch(b):
                    with nc.allow_low_precision(
                        reason="bf16 1/denominator matches baseline attn bf16"
                    ):
                        nc.vector.reciprocal(
                            recip_sb[:, b, :], den_sb[:, b, :]
                        )
                    for ridx in range(4):
                        hh = ridx // 2
                        h = ridx % 2
                        qf = slice(64 * hh, 64 * hh + 64)
                        cols = slice(b * TQ + 512 * h, b * TQ + 512 * (h + 1))
                        ps_b = pb_ctx.tile(
                            [DV, 512], f32, tag="ps_ctx", name=f"ps_b{b}_{ridx}"
                        )
                        nc.tensor.matmul(
                            ps_b,
                            sel_bf[:, ridx * DV : (ridx + 1) * DV],
                            recip_sb[:, b, :],
                            start=True,
                            stop=True,
                        )
                        bcast = pb_bc.tile(
                            [128, 512], bf16, tag="bcast", name=f"bc{b}_{ridx}"
                        )
                        nc.scalar.copy(out=bcast[qf, :], in_=ps_b)
                        nc.vector.tensor_mul(
                            out=ctx_sb[qf, cols],
                            in0=ctx_sb[qf, cols],
                            in1=bcast[qf, :],
                        )

                def exchange_batch(b):
                    # ship this batch's ctx^T; chunk j (128 q cols) -> core j
                    nc.sync.dma_start(
                        out=a2a_in[b][:].rearrange("(j p) t -> p j t", p=FPC),
                        in_=ctx_sb[:, b * TQ : (b + 1) * TQ].rearrange(
                            "p (j t) -> p j t", t=TQ // 8
                        ),
                    )
                    if n_cores > 1:
                        nc.gpsimd.collective_compute(
                            "AllToAll",
                            ALU.bypass,
                            replica_groups=[list(range(n_cores))],
                            ins=[a2a_in[b][:]],
                            outs=[a2a_out[b][:]],
                        )
                    else:
                        # single-core profiling variant: plain copy instead
                        nc.sync.dma_start(out=a2a_out[b][:], in_=a2a_in[b][:])

                pc = tc.alloc_tile_pool(name="pc", bufs=3)
                pc_ps = tc.alloc_tile_pool(name="pc_ps", bufs=2, space="PSUM")
                pc_small = tc.alloc_tile_pool(name="pc_small", bufs=4)

                def phase_c_mc(mc):
                    # output projection + residual + LayerNorm for this
                    # core's 128 q rows of batch mc
                    ps_o = [
                        pc_ps.tile([128, 512], f32, tag="ps_o",
                                   name=f"ps_o{mc}_{nn_}")
                        for nn_ in range(2)
                    ]
                    for kc in range(D // 128):
                        lhs = pc.tile([128, 128], pdt, tag="octx")
                        nc.sync.dma_start(
                            out=lhs,
                            in_=a2a_out[mc][kc * 128 : (kc + 1) * 128, :],
                        )
                        for nn in range(2):
                            nc.tensor.matmul(
                                ps_o[nn],
                                lhs,
                                wo_sb[:, kc, nn * 512 : (nn + 1) * 512],
                                start=(kc == 0),
                                stop=(kc == D // 128 - 1),
                            )
                    o_sb = pc.tile([128, D], f32, tag="o_sb")
                    for nn in range(2):
                        nc.vector.tensor_add(
                            out=o_sb[:, nn * 512 : (nn + 1) * 512],
                            in0=ps_o[nn],
                            in1=qres_sb[:, mc, nn * 512 : (nn + 1) * 512],
                        )
                    # LayerNorm over the free (feature) dim
                    stats = pc_small.tile([128, 2, 6], f32, tag="stats")
                    for sg in range(2):
                        nc.vector.bn_stats(
                            out=stats[:, sg, :], in_=o_sb[:, sg * 512 : (sg + 1) * 512]
                        )
                    mv = pc_small.tile([128, 2], f32, tag="mv")
                    nc.vector.bn_aggr(out=mv, in_=stats)
                    mean, var = mv[:, 0:1], mv[:, 1:2]
                    xve = pc_small.tile([128, 1], f32, tag="xve")
                    nc.vector.tensor_scalar_add(out=xve, in0=var, scalar1=eps_sb)
                    std = pc_small.tile([128, 1], f32, tag="std")
                    nc.scalar.activation(out=std, in_=var, func=Sqrt, bias=eps_sb)
                    rstd = pc_small.tile([128, 1], f32, tag="rstd")
                    nc.vector.reciprocal(rstd, std)
                    # one Newton step for rsqrt accuracy:
                    # r <- r * (1.5 - 0.5 * x * r^2)
                    tnw = pc_small.tile([128, 1], f32, tag="tnw")
                    nc.vector.tensor_mul(out=tnw, in0=rstd, in1=rstd)
                    nc.vector.tensor_mul(out=tnw, in0=tnw, in1=xve)
                    nc.vector.tensor_scalar(
                        out=tnw, in0=tnw, scalar1=-0.5, scalar2=1.5,
                        op0=ALU.mult, op1=ALU.add,
                    )
                    nc.vector.tensor_scalar_mul(out=rstd, in0=rstd, scalar1=tnw)
                    nc.vector.tensor_scalar(
                        out=o_sb, in0=o_sb, scalar1=mean, scalar2=rstd,
                        op0=ALU.subtract, op1=ALU.mult,
                    )
                    nc.vector.tensor_mul(out=o_sb, in0=o_sb, in1=gamma_sb)
                    nc.vector.tensor_add(out=o_sb, in0=o_sb, in1=beta_sb)
                    nc.sync.dma_start(
                        out=out[mc * 128 : (mc + 1) * 128, :], in_=o_sb
                    )

                pairs = [(0, 0, 0), (1, 0, 1), (2, 1, 0), (3, 1, 1)]
                pending = None
                for idx, (pi, b, hh) in enumerate(pairs):
                    if idx == 0:
                        for t in range(4):
                            for th in bd_raw_stages(pi, b, hh, t):
                                th()
                    if idx == 3:
                        # batch-0 output projection overlaps batch-1 attention
                        phase_c_mc(0)
                    fill0 = []
                    for t in range(4):
                        fill0 += bd_raw_stages(pi, b, hh, 4 + t)
                    pending = attn_half(pi, b, hh, 0, fill0, pending)
                    fill1 = []
                    if idx + 1 < 4:
                        pj, bj, hj = pairs[idx + 1]
                        for t in range(4):
                            fill1 += bd_raw_stages(pj, bj, hj, t)
                    pending = attn_half(pi, b, hh, 1, fill1, pending)
                    if idx == 1 or idx == 3:
                        pending()
                        pending = None
                        normalize_batch(b)
                        exchange_batch(b)

                phase_c_mc(1)

                pc_small.release()
                pc_ps.release()
                pc.release()
                pb_ctx.release()
                pb_ps2.release()
                pb_ps.release()
                pb_small.release()
                pb_bc.release()
                pb_prod.release()
                pb_eac.release()
                pb_bdt.release()
                pb_rows.release()
    return nc


def _make_in_maps(inputs, mm_dtype="bfloat16"):
    query = np.asarray(inputs["query"], np.float32)
    key_value = np.asarray(inputs["key_value"], np.float32)
    relative = np.asarray(inputs["relative"], np.float32)
    content_bias = np.asarray(inputs["content_bias"], np.float32)
    position_bias = np.asarray(inputs["position_bias"], np.float32)
    Wq, Wk = np.asarray(inputs["Wq"], np.float32), np.asarray(inputs["Wk"], np.float32)
    Wv, Wr = np.asarray(inputs["Wv"], np.float32), np.asarray(inputs["Wr"], np.float32)
    Wo = np.ascontiguousarray(np.asarray(inputs["Wo"], np.float32))
    ln_gamma = np.asarray(inputs["ln_gamma"], np.float32)
    ln_beta = np.asarray(inputs["ln_beta"], np.float32)

    qflat = query.reshape(B * TQ, D)
    import ml_dtypes

    mdt = ml_dtypes.bfloat16
    f8dt = ml_dtypes.float8_e4m3fn
    xqT = np.ascontiguousarray(qflat.T).astype(f8dt)
    xkvT = np.ascontiguousarray(key_value.reshape(B * TKV, D).T).astype(f8dt)
    xrT = np.ascontiguousarray(relative.reshape(B * TKV, D).T).astype(f8dt)
    Wq, Wk = Wq.astype(f8dt), Wk.astype(f8dt)
    Wv, Wr = Wv.astype(f8dt), Wr.astype(f8dt)
    Wo = Wo.astype(mdt)
    cb = content_bias.reshape(NH, DV)
    pb = position_bias.reshape(NH, DV)

    in_maps = []
    for c in range(N_CORES):
        fs = slice(FPC * c, FPC * (c + 1))
        in_maps.append(
            {
                "xqT": xqT,
                "xkvT": xkvT,
                "xrT": xrT,
                "wq": np.ascontiguousarray(Wq[:, fs]),
                "wk": np.ascontiguousarray(Wk[:, fs]),
                "wv": np.ascontiguousarray(Wv[:, fs]),
                "wr": np.ascontiguousarray(Wr[:, fs]),
                "wo": Wo,
                "cbv": np.ascontiguousarray(
                    cb[HPC * c : HPC * (c + 1)].reshape(FPC, 1)
                ),
                "pbv": np.ascontiguousarray(
                    pb[HPC * c : HPC * (c + 1)].reshape(FPC, 1)
                ),
                "qres": np.ascontiguousarray(
                    np.concatenate(
                        [
                            qflat[128 * c : 128 * (c + 1)],
                            qflat[TQ + 128 * c : TQ + 128 * (c + 1)],
                        ]
                    )
                ),
                "gamma": ln_gamma,
                "beta": ln_beta,
            }
        )
    return in_maps


def run_on_hw(inputs, trace=False, score_dtype="bfloat16", proj_dtype="bfloat16"):
    from concourse.bass_utils import run_bass_kernel_spmd

    key = (score_dtype, proj_dtype)
    nc = _CACHE.get(key)
    if nc is None:
        nc = build_program(score_dtype=score_dtype, proj_dtype=proj_dtype)
        _CACHE[key] = nc
    in_maps = _make_in_maps(inputs, mm_dtype=proj_dtype)
    res = run_bass_kernel_spmd(nc, in_maps, list(range(N_CORES)), trace=trace)
    full = np.empty((B * TQ, D), np.float32)
    for c in range(N_CORES):
        o = np.asarray(res.results[c]["out"])
        full[128 * c : 128 * (c + 1)] = o[:128]
        full[TQ + 128 * c : TQ + 128 * (c + 1)] = o[128:]
    return full.reshape(B, TQ, D), res


def kernel(**inputs) -> np.ndarray:
    out, _ = run_on_hw(inputs)
    return out
